# revision 6
# baseline (speedup 1.0000x reference)
"""Trainium2 Bass kernel for the CustomLossFilter loss.

reference semantics (per row, fp32):
    cond = |inputs[:,4] - inputs[:,2]| < 0.1
    diff = where(cond, inputs[:,0] - inputs[:,4], inputs[:,0] - targets[:,0])
    out  = mean(|diff|)

Strategy: data-parallel over the 20M rows across 8 NeuronCores (2.5M rows
per core).  Inside a core, rows are mapped [128 partitions x 19531 rows]
with each partition owning a contiguous row range, so every DMA is a plain
contiguous 2D transfer.  Columns 0/2/4 are accessed with stride-5 APs in
SBUF.  The kernel is HBM-bound (60 MB/core at ~358 GB/s ~= 168 us), so the
compute chain is spread across GpSimd (cond subtract), Vector (mask +
select + diff) and Scalar (abs+accumulate) to keep every engine well under
the DMA budget, and the trailing tiles taper down so the pipeline drains
quickly after the last DMA.  Each core emits a [128,1] vector of
per-partition |diff| sums; the host adds the 1024 partials and divides by N.
"""

import numpy as np

import concourse.bacc as bacc
import concourse.mybir as mybir
from concourse import tile
from concourse.bass_utils import run_bass_kernel_spmd

N_TOTAL = 20_000_000
F = 5
N_CORES = 8
ROWS = N_TOTAL // N_CORES  # 2_500_000 rows per core
P = 128
W = 2048  # rows per partition per main tile
ERR_OK = 0.1

_ALU = mybir.AluOpType
_AX = mybir.AxisListType
_F32 = mybir.dt.float32
_U8 = mybir.dt.uint8
_ABS = mybir.ActivationFunctionType.Abs


def _widths(rpp, w):
    """Main tiles of width w, remainder tapered (~55% chunks) so the last
    tile is small and the post-DMA drain is short."""
    widths = []
    rem = rpp
    while rem > w:
        widths.append(w)
        rem -= w
    while rem > 384:
        c = (rem * 11) // 20
        widths.append(c)
        rem -= c
    if rem:
        widths.append(rem)
    return widths


def _body(tc, inp, tgt, out, rows, w):
    nc = tc.nc
    rpp = rows // P          # rows per partition in the main region
    scrap = rows - P * rpp   # leftover rows (< 128)

    widths = _widths(rpp, w)
    nt = len(widths) + (1 if scrap else 0)

    # [128, rpp*5] / [128, rpp] contiguous-per-partition views of DRAM
    in_main = inp[: P * rpp, :].rearrange("(p r) f -> p (r f)", p=P)
    tg_main = tgt[: P * rpp, :].rearrange("(p r) f -> p (r f)", p=P)

    with (
        tc.tile_pool(name="acc", bufs=1) as accpool,
        tc.tile_pool(name="inp", bufs=3) as inpool,
        tc.tile_pool(name="tgp", bufs=3) as tgpool,
        tc.tile_pool(name="dfp", bufs=2) as dpool,
        tc.tile_pool(name="abp", bufs=2) as apool,
        tc.tile_pool(name="msk", bufs=2) as mpool,
    ):
        acc = accpool.tile([P, nt], _F32)
        nc.gpsimd.memset(acc[:], 0.0)

        off = 0
        for t, wt in enumerate(widths):
            ti = inpool.tile([P, w * F], _F32, tag="in")
            tt = tgpool.tile([P, w], _F32, tag="tg")
            nc.sync.dma_start(ti[:, : wt * F], in_main[:, off * F : (off + wt) * F])
            nc.scalar.dma_start(tt[:, :wt], tg_main[:, off : off + wt])

            in0 = ti[:, 0 : wt * F : F]
            in2 = ti[:, 2 : wt * F : F]
            in4 = ti[:, 4 : wt * F : F]

            d = dpool.tile([P, w], _F32, tag="d")
            a = apool.tile([P, w], _F32, tag="ab")
            m = mpool.tile([P, w], _U8, tag="m")
            diff = dpool.tile([P, w], _F32, tag="f")
            adiff = apool.tile([P, w], _F32, tag="ab")  # write-only scratch
            # cond subtract on the (otherwise idle) GpSimd engine
            nc.gpsimd.tensor_tensor(d[:, :wt], in4, in2, _ALU.subtract)
            nc.scalar.activation(a[:, :wt], d[:, :wt], _ABS)
            nc.vector.tensor_scalar(m[:, :wt], a[:, :wt], ERR_OK, None, _ALU.is_lt)
            nc.vector.copy_predicated(tt[:, :wt], m[:, :wt], in4)
            nc.vector.tensor_tensor(diff[:, :wt], in0, tt[:, :wt], _ALU.subtract)
            nc.scalar.activation(
                adiff[:, :wt], diff[:, :wt], _ABS, accum_out=acc[:, t : t + 1]
            )
            off += wt

            if t == 0 and scrap:
                # tiny leftover block: emit early so it never sits in the
                # pipeline tail
                si = inpool.tile([scrap, F], _F32, tag="sin")
                st = tgpool.tile([scrap, 1], _F32, tag="stg")
                nc.sync.dma_start(si[:], inp[P * rpp :, :])
                nc.scalar.dma_start(st[:], tgt[P * rpp :, :])
                sd = dpool.tile([scrap, 1], _F32, tag="sd")
                sb = apool.tile([scrap, 1], _F32, tag="sb")
                sm = mpool.tile([scrap, 1], _U8, tag="sm")
                sa = apool.tile([scrap, 1], _F32, tag="sb")
                nc.gpsimd.tensor_tensor(sd[:], si[:, 4:5], si[:, 2:3], _ALU.subtract)
                nc.scalar.activation(sb[:], sd[:], _ABS)
                nc.vector.tensor_scalar(sm[:], sb[:], ERR_OK, None, _ALU.is_lt)
                nc.vector.copy_predicated(st[:], sm[:], si[:, 4:5])
                sdiff = dpool.tile([scrap, 1], _F32, tag="sd")
                nc.vector.tensor_tensor(sdiff[:], si[:, 0:1], st[:], _ALU.subtract)
                nc.scalar.activation(
                    sa[:], sdiff[:], _ABS, accum_out=acc[:scrap, nt - 1 : nt]
                )

        res = accpool.tile([P, 1], _F32)
        nc.vector.tensor_reduce(res[:], acc[:], axis=_AX.X, op=_ALU.add)
        nc.sync.dma_start(out[:], res[:])


def build_nc(rows=ROWS, w=W):
    nc = bacc.Bacc(
        "TRN2", target_bir_lowering=False, debug=False, num_devices=N_CORES
    )
    inp = nc.dram_tensor("inputs", [rows, F], _F32, kind="ExternalInput").ap()
    tgt = nc.dram_tensor("targets", [rows, 1], _F32, kind="ExternalInput").ap()
    out = nc.dram_tensor("out", [P, 1], _F32, kind="ExternalOutput").ap()
    with tile.TileContext(nc) as tc:
        _body(tc, inp, tgt, out, rows, w)
    nc.compile()
    return nc


_NC_CACHE = {}


def _get_nc():
    if "nc" not in _NC_CACHE:
        _NC_CACHE["nc"] = build_nc()
    return _NC_CACHE["nc"]


def run_sharded(inputs, targets, **spmd_kwargs):
    """Run the SPMD kernel; returns (per-core [128,1] partials, results obj)."""
    nc = _get_nc()
    inputs = np.asarray(inputs, dtype=np.float32)
    targets = np.asarray(targets, dtype=np.float32)
    in_maps = [
        {
            "inputs": inputs[i * ROWS : (i + 1) * ROWS],
            "targets": targets[i * ROWS : (i + 1) * ROWS],
        }
        for i in range(N_CORES)
    ]
    res = run_bass_kernel_spmd(nc, in_maps, list(range(N_CORES)), **spmd_kwargs)
    partials = np.stack([r["out"] for r in res.results])  # [8, 128, 1]
    return partials, res


def kernel(inputs, targets):
    partials, _ = run_sharded(inputs, targets)
    total = partials.astype(np.float64).sum()
    return np.asarray(total / N_TOTAL, dtype=np.float32)


# revision 9
# speedup vs baseline: 1.0738x; 1.0738x over previous
"""Trainium2 Bass kernel for the CustomLossFilter loss.

reference semantics (per row, fp32):
    cond = |inputs[:,4] - inputs[:,2]| < 0.1
    diff = where(cond, inputs[:,0] - inputs[:,4], inputs[:,0] - targets[:,0])
    out  = mean(|diff|)

Strategy: data-parallel over the 20M rows across 8 NeuronCores (2.5M rows
per core).  Inside a core, rows are mapped [128 partitions x 19531 rows]
with each partition owning a contiguous row range, so every DMA is a plain
contiguous 2D transfer.  Columns 0/2/4 are accessed with stride-5 APs in
SBUF.  The kernel is HBM-bound (60 MB/core at ~358 GB/s ~= 168 us), so the
compute chain is spread across GpSimd (cond subtract), Vector (mask +
select + diff) and Scalar (abs+accumulate) to keep every engine well under
the DMA budget, and the trailing tiles taper down so the pipeline drains
quickly after the last DMA.  Each core emits a [128,1] vector of
per-partition |diff| sums; the host adds the 1024 partials and divides by N.
"""

import numpy as np

import concourse.bacc as bacc
import concourse.mybir as mybir
from concourse import tile
from concourse.bass_utils import run_bass_kernel_spmd

N_TOTAL = 20_000_000
F = 5
N_CORES = 8
ROWS = N_TOTAL // N_CORES  # 2_500_000 rows per core
P = 128
W = 2048  # rows per partition per main tile
ERR_OK = 0.1

_ALU = mybir.AluOpType
_AX = mybir.AxisListType
_F32 = mybir.dt.float32
_U8 = mybir.dt.uint8
_ABS = mybir.ActivationFunctionType.Abs


def _widths(rpp, w):
    """Small leading tiles (fast pipeline fill), main tiles of width w, and
    a split remainder so the last tile is small and the post-DMA drain is
    short.  All tiles stay >=1MB DMAs to keep descriptor efficiency."""
    widths = []
    rem = rpp
    for c in (w // 4, w // 2):
        if rem > 3 * c:
            widths.append(c)
            rem -= c
    while rem > w:
        widths.append(w)
        rem -= w
    if rem > (w * 3) // 4:
        c = (rem * 5) // 7
        widths.append(c)
        rem -= c
    if rem:
        widths.append(rem)
    return widths


def _body(tc, inp, tgt, out, rows, w):
    nc = tc.nc
    rpp = rows // P          # rows per partition in the main region
    scrap = rows - P * rpp   # leftover rows (< 128)

    widths = _widths(rpp, w)
    nt = len(widths) + (1 if scrap else 0)

    # [128, rpp*5] / [128, rpp] contiguous-per-partition views of DRAM
    in_main = inp[: P * rpp, :].rearrange("(p r) f -> p (r f)", p=P)
    tg_main = tgt[: P * rpp, :].rearrange("(p r) f -> p (r f)", p=P)

    with (
        tc.tile_pool(name="acc", bufs=1) as accpool,
        tc.tile_pool(name="inp", bufs=3) as inpool,
        tc.tile_pool(name="tgp", bufs=3) as tgpool,
        tc.tile_pool(name="dfp", bufs=2) as dpool,
        tc.tile_pool(name="abp", bufs=2) as apool,
        tc.tile_pool(name="msk", bufs=2) as mpool,
    ):
        acc = accpool.tile([P, nt], _F32)
        nc.gpsimd.memset(acc[:], 0.0)

        off = 0
        for t, wt in enumerate(widths):
            ti = inpool.tile([P, w * F], _F32, tag="in")
            tt = tgpool.tile([P, w], _F32, tag="tg")
            nc.sync.dma_start(ti[:, : wt * F], in_main[:, off * F : (off + wt) * F])
            nc.scalar.dma_start(tt[:, :wt], tg_main[:, off : off + wt])

            in0 = ti[:, 0 : wt * F : F]
            in2 = ti[:, 2 : wt * F : F]
            in4 = ti[:, 4 : wt * F : F]

            d = dpool.tile([P, w], _F32, tag="d")
            a = apool.tile([P, w], _F32, tag="ab")
            m = mpool.tile([P, w], _U8, tag="m")
            diff = dpool.tile([P, w], _F32, tag="f")
            adiff = apool.tile([P, w], _F32, tag="ab")  # write-only scratch
            # whole cond chain on the (otherwise idle) GpSimd engine:
            # |in4-in2| < 0.1  <=>  (in4-in2)^2 < 0.01.  Keeps the Scalar
            # engine out of the mask chain so its in-order queue (which also
            # carries the final abs+accum) never serializes the pipeline.
            nc.gpsimd.tensor_tensor(d[:, :wt], in4, in2, _ALU.subtract)
            nc.gpsimd.tensor_tensor(a[:, :wt], d[:, :wt], d[:, :wt], _ALU.mult)
            nc.vector.tensor_scalar(
                m[:, :wt], a[:, :wt], ERR_OK * ERR_OK, None, _ALU.is_lt
            )
            nc.vector.copy_predicated(tt[:, :wt], m[:, :wt], in4)
            nc.vector.tensor_tensor(diff[:, :wt], in0, tt[:, :wt], _ALU.subtract)
            nc.scalar.activation(
                adiff[:, :wt], diff[:, :wt], _ABS, accum_out=acc[:, t : t + 1]
            )
            off += wt

            if t == 0 and scrap:
                # tiny leftover block: emit early so it never sits in the
                # pipeline tail
                si = inpool.tile([scrap, F], _F32, tag="sin")
                st = tgpool.tile([scrap, 1], _F32, tag="stg")
                nc.sync.dma_start(si[:], inp[P * rpp :, :])
                nc.scalar.dma_start(st[:], tgt[P * rpp :, :])
                sd = dpool.tile([scrap, 1], _F32, tag="sd")
                sb = apool.tile([scrap, 1], _F32, tag="sb")
                sm = mpool.tile([scrap, 1], _U8, tag="sm")
                sa = apool.tile([scrap, 1], _F32, tag="sb")
                nc.gpsimd.tensor_tensor(sd[:], si[:, 4:5], si[:, 2:3], _ALU.subtract)
                nc.gpsimd.tensor_tensor(sb[:], sd[:], sd[:], _ALU.mult)
                nc.vector.tensor_scalar(
                    sm[:], sb[:], ERR_OK * ERR_OK, None, _ALU.is_lt
                )
                nc.vector.copy_predicated(st[:], sm[:], si[:, 4:5])
                sdiff = dpool.tile([scrap, 1], _F32, tag="sd")
                nc.vector.tensor_tensor(sdiff[:], si[:, 0:1], st[:], _ALU.subtract)
                nc.scalar.activation(
                    sa[:], sdiff[:], _ABS, accum_out=acc[:scrap, nt - 1 : nt]
                )

        res = accpool.tile([P, 1], _F32)
        nc.vector.tensor_reduce(res[:], acc[:], axis=_AX.X, op=_ALU.add)
        nc.sync.dma_start(out[:], res[:])


def build_nc(rows=ROWS, w=W):
    nc = bacc.Bacc(
        "TRN2", target_bir_lowering=False, debug=False, num_devices=N_CORES
    )
    inp = nc.dram_tensor("inputs", [rows, F], _F32, kind="ExternalInput").ap()
    tgt = nc.dram_tensor("targets", [rows, 1], _F32, kind="ExternalInput").ap()
    out = nc.dram_tensor("out", [P, 1], _F32, kind="ExternalOutput").ap()
    with tile.TileContext(nc) as tc:
        _body(tc, inp, tgt, out, rows, w)
    nc.compile()
    return nc


_NC_CACHE = {}


def _get_nc():
    if "nc" not in _NC_CACHE:
        _NC_CACHE["nc"] = build_nc()
    return _NC_CACHE["nc"]


def run_sharded(inputs, targets, **spmd_kwargs):
    """Run the SPMD kernel; returns (per-core [128,1] partials, results obj)."""
    nc = _get_nc()
    inputs = np.asarray(inputs, dtype=np.float32)
    targets = np.asarray(targets, dtype=np.float32)
    in_maps = [
        {
            "inputs": inputs[i * ROWS : (i + 1) * ROWS],
            "targets": targets[i * ROWS : (i + 1) * ROWS],
        }
        for i in range(N_CORES)
    ]
    res = run_bass_kernel_spmd(nc, in_maps, list(range(N_CORES)), **spmd_kwargs)
    partials = np.stack([r["out"] for r in res.results])  # [8, 128, 1]
    return partials, res


def kernel(inputs, targets):
    partials, _ = run_sharded(inputs, targets)
    total = partials.astype(np.float64).sum()
    return np.asarray(total / N_TOTAL, dtype=np.float32)


# revision 15
# speedup vs baseline: 1.2932x; 1.2043x over previous
"""Trainium2 Bass kernel for the CustomLossFilter loss.

reference semantics (per row, fp32):
    cond = |inputs[:,4] - inputs[:,2]| < 0.1
    diff = where(cond, inputs[:,0] - inputs[:,4], inputs[:,0] - targets[:,0])
    out  = mean(|diff|)

Strategy: data-parallel over the 20M rows across 8 NeuronCores (2.5M rows
per core).  Inside a core, rows are mapped [128 partitions x 19531 rows]
with each partition owning a contiguous row range, so every DMA is a plain
contiguous 2D transfer.  Columns 0/2/4 are accessed with stride-5 APs in
SBUF.  The kernel is HBM-bound (60 MB/core at ~358 GB/s ~= 168 us), so the
compute chain is spread across GpSimd (cond subtract), Vector (mask +
select + diff) and Scalar (abs+accumulate) to keep every engine well under
the DMA budget, and the trailing tiles taper down so the pipeline drains
quickly after the last DMA.  Each core emits a [128,1] vector of
per-partition |diff| sums; the host adds the 1024 partials and divides by N.
"""

import numpy as np

import concourse.bacc as bacc
import concourse.mybir as mybir
from concourse import tile
from concourse.bass_utils import run_bass_kernel_spmd

N_TOTAL = 20_000_000
F = 5
N_CORES = 8
ROWS = N_TOTAL // N_CORES  # 2_500_000 rows per core
P = 128
W = 2048  # rows per partition per main tile
ERR_OK = 0.1

_ALU = mybir.AluOpType
_AX = mybir.AxisListType
_F32 = mybir.dt.float32
_U8 = mybir.dt.uint8
_ABS = mybir.ActivationFunctionType.Abs


def _widths(rpp, w):
    """Small leading tiles (fast pipeline fill), main tiles of width w, and
    a split remainder so the last tile is small and the post-DMA drain is
    short.  All tiles stay >=1MB DMAs to keep descriptor efficiency."""
    widths = []
    rem = rpp
    for c in (w // 4, w // 2):
        if rem > 3 * c:
            widths.append(c)
            rem -= c
    while rem > w:
        widths.append(w)
        rem -= w
    while rem > (w * 5) // 16:
        widths.append((w * 5) // 16)
        rem -= (w * 5) // 16
    if rem:
        widths.append(rem)
    return widths


def _body(tc, inp, tgt, out, rows, w):
    nc = tc.nc
    rpp = rows // P          # rows per partition in the main region
    scrap = rows - P * rpp   # leftover rows (< 128)

    widths = _widths(rpp, w)
    nt = len(widths) + (1 if scrap else 0)

    # [128, rpp*5] / [128, rpp] contiguous-per-partition views of DRAM
    in_main = inp[: P * rpp, :].rearrange("(p r) f -> p (r f)", p=P)
    tg_main = tgt[: P * rpp, :].rearrange("(p r) f -> p (r f)", p=P)

    with (
        tc.tile_pool(name="acc", bufs=1) as accpool,
        tc.tile_pool(name="inp", bufs=3) as inpool,
        tc.tile_pool(name="tgp", bufs=3) as tgpool,
        tc.tile_pool(name="dfp", bufs=2) as dpool,
        tc.tile_pool(name="abp", bufs=2) as apool,
        tc.tile_pool(name="msk", bufs=2) as mpool,
    ):
        acc = accpool.tile([P, nt], _F32)
        nc.gpsimd.memset(acc[:], 0.0)

        off = 0
        for t, wt in enumerate(widths):
            ti = inpool.tile([P, w * F], _F32, tag="in")
            tt = tgpool.tile([P, w], _F32, tag="tg")
            nc.sync.dma_start(ti[:, : wt * F], in_main[:, off * F : (off + wt) * F])
            nc.scalar.dma_start(tt[:, :wt], tg_main[:, off : off + wt])

            in0 = ti[:, 0 : wt * F : F]
            in2 = ti[:, 2 : wt * F : F]
            in4 = ti[:, 4 : wt * F : F]

            d = dpool.tile([P, w], _F32, tag="d")
            m = mpool.tile([P, w], _U8, tag="m")
            diff = dpool.tile([P, w], _F32, tag="f")
            adiff = apool.tile([P, w], _F32, tag="ab")  # write-only scratch
            # cond subtract on the (otherwise idle) GpSimd engine, keeping
            # the Scalar engine out of the mask chain so its in-order queue
            # (which also carries the final abs+accum) never serializes the
            # pipeline.  |d| < 0.1f computed exactly in int space in one DVE
            # op: clear the sign bit, then unsigned-compare against the bit
            # pattern of 0.1f (positive IEEE754 floats order like ints).
            du = d[:, :wt].bitcast(mybir.dt.uint32)
            nc.gpsimd.tensor_tensor(d[:, :wt], in4, in2, _ALU.subtract)
            nc.vector.tensor_scalar(du, du, 0x7FFFFFFF, None, _ALU.bitwise_and)
            nc.vector.tensor_scalar(m[:, :wt], du, 0x3DCCCCCD, None, _ALU.is_lt)
            nc.vector.copy_predicated(tt[:, :wt], m[:, :wt], in4)
            nc.vector.tensor_tensor(diff[:, :wt], in0, tt[:, :wt], _ALU.subtract)
            nc.scalar.activation(
                adiff[:, :wt], diff[:, :wt], _ABS, accum_out=acc[:, t : t + 1]
            )
            off += wt

            if t == 0 and scrap:
                # tiny leftover block: emit early so it never sits in the
                # pipeline tail
                si = inpool.tile([scrap, F], _F32, tag="sin")
                st = tgpool.tile([scrap, 1], _F32, tag="stg")
                nc.sync.dma_start(si[:], inp[P * rpp :, :])
                nc.scalar.dma_start(st[:], tgt[P * rpp :, :])
                sd = dpool.tile([scrap, 1], _F32, tag="sd")
                sm = mpool.tile([scrap, 1], _U8, tag="sm")
                sa = apool.tile([scrap, 1], _F32, tag="sb")
                sdu = sd[:].bitcast(mybir.dt.uint32)
                nc.gpsimd.tensor_tensor(sd[:], si[:, 4:5], si[:, 2:3], _ALU.subtract)
                nc.vector.tensor_scalar(sdu, sdu, 0x7FFFFFFF, None, _ALU.bitwise_and)
                nc.vector.tensor_scalar(sm[:], sdu, 0x3DCCCCCD, None, _ALU.is_lt)
                nc.vector.copy_predicated(st[:], sm[:], si[:, 4:5])
                sdiff = dpool.tile([scrap, 1], _F32, tag="sd")
                nc.vector.tensor_tensor(sdiff[:], si[:, 0:1], st[:], _ALU.subtract)
                nc.scalar.activation(
                    sa[:], sdiff[:], _ABS, accum_out=acc[:scrap, nt - 1 : nt]
                )

        res = accpool.tile([P, 1], _F32)
        nc.vector.tensor_reduce(res[:], acc[:], axis=_AX.X, op=_ALU.add)
        nc.sync.dma_start(out[:], res[:])


def build_nc(rows=ROWS, w=W):
    nc = bacc.Bacc(
        "TRN2", target_bir_lowering=False, debug=False, num_devices=N_CORES
    )
    inp = nc.dram_tensor("inputs", [rows, F], _F32, kind="ExternalInput").ap()
    tgt = nc.dram_tensor("targets", [rows, 1], _F32, kind="ExternalInput").ap()
    out = nc.dram_tensor("out", [P, 1], _F32, kind="ExternalOutput").ap()
    with tile.TileContext(nc) as tc:
        _body(tc, inp, tgt, out, rows, w)
    nc.compile()
    return nc


_NC_CACHE = {}


def _get_nc():
    if "nc" not in _NC_CACHE:
        _NC_CACHE["nc"] = build_nc()
    return _NC_CACHE["nc"]


def run_sharded(inputs, targets, **spmd_kwargs):
    """Run the SPMD kernel; returns (per-core [128,1] partials, results obj)."""
    nc = _get_nc()
    inputs = np.asarray(inputs, dtype=np.float32)
    targets = np.asarray(targets, dtype=np.float32)
    in_maps = [
        {
            "inputs": inputs[i * ROWS : (i + 1) * ROWS],
            "targets": targets[i * ROWS : (i + 1) * ROWS],
        }
        for i in range(N_CORES)
    ]
    res = run_bass_kernel_spmd(nc, in_maps, list(range(N_CORES)), **spmd_kwargs)
    partials = np.stack([r["out"] for r in res.results])  # [8, 128, 1]
    return partials, res


def kernel(inputs, targets):
    partials, _ = run_sharded(inputs, targets)
    total = partials.astype(np.float64).sum()
    return np.asarray(total / N_TOTAL, dtype=np.float32)
